# revision 2
# baseline (speedup 1.0000x reference)
"""BatchMultiHeadGraphAttention Trainium2 kernel (8 NeuronCores, SPMD).

Reference computation (per batch b, head h):
    hp   = h[b] @ w[h]                      [n, 64]
    t    = tanh(hp)
    src  = t @ a_src[h];  dst = t @ a_dst[h]        [n]
    attn = softmax_j( leaky_relu(src_i + dst_j, 0.2) )
    out  = attn @ hp + bias

Key identity used here: with z = src_i + dst_j,
    exp(lrelu(z)) = max(exp(z), exp(0.2 z))            (monotonicity)
                  = e02s_i * e02d_j * max(q_i * r_j, 1)
with q = exp(0.8 src), r = exp(0.8 dst), e02d = exp(0.2 dst).
The e02s_i factor cancels in the softmax normalization, and e02d_j folds
into the matmul stationary operand, so the whole [n, n] attention matrix
costs ONE fused DVE pass (mult+max) per 128-row chunk:
    g[j, i]   = max(q_i * r_j, 1)                       (bf16)
    psum[o,i] = sum_j (e02d_j * (hp[j,o] + bias_o)) * g[j,i]   (PE, bf16)
    psum[64,i]= sum_j  e02d_j * g[j,i]                  (ones column)
    out_T[o,i]= psum[o,i] / psum[64,i]
bias is folded exactly: sum_j p*(hp+bias)/sum_j p = out + bias.

Sharding: 16 (b,h) pairs over 8 cores -> core c handles batch c//2,
heads {2*(c%2), 2*(c%2)+1}. Output returned transposed [64, n] per head;
the host does the final [o, n] -> [n, o] transpose during the gather.
"""

import numpy as np

import concourse.bass as bass
import concourse.mybir as mybir
from concourse.tile import TileContext
from concourse.bass_utils import run_bass_kernel_spmd
from concourse.masks import make_identity

F32 = mybir.dt.float32
F32R = mybir.dt.float32r
BF16 = mybir.dt.bfloat16
AF = mybir.ActivationFunctionType
ALU = mybir.AluOpType

N_HEAD, F_IN, F_OUT = 4, 768, 64
BS, N = 4, 2048
NCORES = 8
NCH = N // 128      # 16 n-chunks of 128
FCH = F_IN // 128   # 6 f-chunks of 128
NBL = N // 512      # 4 n-blocks of 512


def _split_multi_waits(nc):
    """walrus in this container accepts at most ONE sync wait per
    instruction; hoist extra waits onto same-engine NOPs."""
    k = 0
    for fn in nc.m.functions:
        for bb in fn.blocks:
            il = bb.instructions
            new_list = []
            for inst in il:
                si = inst.sync_info
                if si is not None and si.on_wait and len(si.on_wait) > 1:
                    waits = list(si.on_wait)
                    for w in waits[:-1]:
                        nop = mybir.InstNoOp(
                            name=f"waitsplit-{k}",
                            engine=inst.engine,
                            sync_info=mybir.SyncInfo(on_wait=[w], on_update=[]),
                            bass_nofuse=True,
                        )
                        k += 1
                        new_list.append(nop)
                    inst.sync_info = mybir.SyncInfo(
                        on_wait=[waits[-1]], on_update=list(si.on_update or [])
                    )
                new_list.append(inst)
            il[:] = new_list
    return k


def build_module():
    nc = bass.Bass("TRN2", target_bir_lowering=False, debug=False,
                   num_devices=NCORES)
    h_d = nc.dram_tensor("h", [N, F_IN], F32, kind="ExternalInput")
    w2_d = nc.dram_tensor("w2", [F_IN, 128], F32, kind="ExternalInput")
    asd_d = nc.dram_tensor("asd", [F_OUT, 4], F32, kind="ExternalInput")
    bias_d = nc.dram_tensor("biasc", [F_OUT, 1], F32, kind="ExternalInput")
    o_d = nc.dram_tensor("o", [2, F_OUT, N], F32, kind="ExternalOutput")

    with TileContext(nc, num_cores=NCORES) as tc:
        with (
            tc.tile_pool(name="const", bufs=1) as const,
            tc.tile_pool(name="big", bufs=1) as big,
            tc.tile_pool(name="hstage", bufs=3) as hstage,
            tc.tile_pool(name="gpool", bufs=3) as gpool,
            tc.tile_pool(name="dram", bufs=1, space="DRAM") as dram,
        ):
            ident = const.tile([128, 128], F32)
            make_identity(nc, ident)
            w2r = const.tile([128, FCH, 128], F32R)
            nc.sync.dma_start(
                w2r[:], w2_d[:].rearrange("(c p) m -> p c m", p=128).bitcast(F32R))
            asd_sb = const.tile([F_OUT, 4], F32)
            nc.sync.dma_start(asd_sb[:], asd_d[:])
            bias_sb = const.tile([F_OUT, 1], F32)
            nc.sync.dma_start(bias_sb[:], bias_d[:])

            # persistent SBUF
            hT = [big.tile([128, N], F32R, tag=f"hT{fc}", name=f"hT{fc}") for fc in range(FCH)]
            hpT = [big.tile([65, N], F32, tag=f"hpT{hh}", name=f"hpT{hh}") for hh in range(2)]
            tT = [big.tile([F_OUT, N], F32, tag=f"tT{hh}", name=f"tT{hh}") for hh in range(2)]

            # ---------- Phase A: load h and transpose to hT (f32r) ----------
            with tc.tile_pool(name="psA", bufs=3, space="PSUM") as psA, \
                 tc.tile_pool(name="psB", bufs=4, space="PSUM") as psB:
                for ncnk in range(NCH):
                    hs = hstage.tile([128, F_IN], F32)
                    nc.sync.dma_start(hs[:], h_d[ncnk * 128:(ncnk + 1) * 128, :])
                    for fc in range(FCH):
                        pt = psA.tile([128, 128], F32, tag="pt")
                        nc.tensor.transpose(
                            pt[:], hs[:, fc * 128:(fc + 1) * 128], ident[:])
                        dst = hT[fc][:, ncnk * 128:(ncnk + 1) * 128]
                        if (ncnk * FCH + fc) % 2 == 0:
                            nc.vector.tensor_copy(dst, pt[:])
                        else:
                            nc.scalar.activation(dst, pt[:], AF.Copy)

                # ---------- Phase B: hp_T for both heads, tanh, bias-fold ----
                for hh in range(2):
                    nc.gpsimd.memset(hpT[hh][64:65, :], 1.0)
                for nb in range(NBL):
                    pB = psB.tile([128, 512], F32, tag="pB")
                    for fc in range(FCH):
                        nc.tensor.matmul(
                            pB[:], w2r[:, fc, :],
                            hT[fc][:, nb * 512:(nb + 1) * 512],
                            start=(fc == 0), stop=(fc == FCH - 1))
                    nbs = slice(nb * 512, (nb + 1) * 512)
                    for hh in range(2):
                        rows = slice(hh * 64, hh * 64 + 64)
                        nc.scalar.activation(tT[hh][:, nbs], pB[rows, :], AF.Tanh)
                        nc.scalar.activation(hpT[hh][0:64, nbs], pB[rows, :],
                                             AF.Identity, bias=bias_sb[:])

            # ---------- per-head phases ----------
            with tc.tile_pool(name="psM", bufs=2, space="PSUM") as psM, \
                 tc.tile_pool(name="psOut", bufs=4, space="PSUM") as psOut:
                for hh in range(2):
                    # Phase C: src/dst rows via PE; q/r/e02d exps
                    sd_sb = big.tile([2, N], F32, tag=f"sd{hh}")
                    for nb in range(NBL):
                        nbs = slice(nb * 512, (nb + 1) * 512)
                        pC = psM.tile([2, 512], F32, tag="psm")
                        nc.tensor.matmul(pC[:], asd_sb[:, 2 * hh:2 * hh + 2],
                                         tT[hh][:, nbs], start=True, stop=True)
                        nc.scalar.activation(sd_sb[:, nbs], pC[:], AF.Copy)
                    q_bf = big.tile([1, N], BF16, tag=f"q{hh}")
                    nc.scalar.activation(q_bf[:], sd_sb[0:1, :], AF.Exp, scale=0.8)
                    qd = dram.tile([1, N], BF16, tag=f"qd{hh}")
                    nc.sync.dma_start(qd[:], q_bf[:])
                    q_bcast = big.tile([128, N], BF16, tag=f"qb{hh}")
                    nc.sync.dma_start(q_bcast[:], qd[:].partition_broadcast(128))

                    pD = psM.tile([128, NCH, 2], F32, tag="psm")
                    for k in range(NCH):
                        nc.tensor.transpose(pD[:, k, :],
                                            sd_sb[:, k * 128:(k + 1) * 128],
                                            ident[0:2, 0:2])
                    r_col = big.tile([128, NCH], F32, tag=f"r{hh}")
                    nc.scalar.activation(r_col[:], pD[:, :, 1], AF.Exp, scale=0.8)
                    e02d = big.tile([128, NCH], F32, tag=f"e02d{hh}")
                    nc.scalar.activation(e02d[:], pD[:, :, 1], AF.Exp, scale=0.2)

                    # Phase D: stationary tiles hpe02X = e02d * [hp + bias | 1]
                    xs = []
                    for jc in range(NCH):
                        pX = psM.tile([128, 65], F32, tag="psm")
                        nc.tensor.transpose(pX[:],
                                            hpT[hh][0:65, jc * 128:(jc + 1) * 128],
                                            ident[0:65, 0:65])
                        xt = big.tile([128, 65], BF16, tag=f"x{hh}_{jc}", name=f"x{hh}_{jc}")
                        nc.vector.tensor_scalar_mul(xt[:], pX[:],
                                                    e02d[:, jc:jc + 1])
                        xs.append(xt)

                    # Phase F: main attention loop
                    pOut = []
                    for ib in range(NBL):
                        pOut.append(psOut.tile([65, 512], F32, tag="pout", name=f"pout{hh}_{ib}"))
                    for jc in range(NCH):
                        g = gpool.tile([128, N], BF16, tag="g")
                        nc.vector.tensor_scalar(
                            out=g[:], in0=q_bcast[:],
                            scalar1=r_col[:, jc:jc + 1], scalar2=1.0,
                            op0=ALU.mult, op1=ALU.max)
                        for ib in range(NBL):
                            nc.tensor.matmul(
                                pOut[ib][:], xs[jc][:],
                                g[:, ib * 512:(ib + 1) * 512],
                                start=(jc == 0), stop=(jc == NCH - 1))

                    # Phase G: normalize + store
                    rs = big.tile([1, N], F32, tag=f"rs{hh}")
                    for ib in range(NBL):
                        nc.vector.reciprocal(rs[0:1, ib * 512:(ib + 1) * 512],
                                             pOut[ib][64:65, :])
                    rsd = dram.tile([1, N], F32, tag=f"rsd{hh}")
                    nc.sync.dma_start(rsd[:], rs[:])
                    rs_bcast = big.tile([F_OUT, N], F32, tag=f"rsb{hh}")
                    nc.sync.dma_start(rs_bcast[:], rsd[:].partition_broadcast(F_OUT))
                    out_sb = big.tile([F_OUT, N], F32, tag=f"out{hh}")
                    for ib in range(NBL):
                        nbs = slice(ib * 512, (ib + 1) * 512)
                        nc.vector.tensor_tensor(
                            out=out_sb[:, nbs], in0=pOut[ib][0:64, :],
                            in1=rs_bcast[:, nbs], op=ALU.mult)
                    nc.sync.dma_start(o_d[hh], out_sb[:])

    _split_multi_waits(nc)
    return nc


_NC_CACHE = None


def kernel(h, w, a_src, a_dst, bias):
    global _NC_CACHE
    if _NC_CACHE is None:
        _NC_CACHE = build_module()
    nc = _NC_CACHE

    h = np.asarray(h, dtype=np.float32)
    w = np.asarray(w, dtype=np.float32)
    a_src = np.asarray(a_src, dtype=np.float32)
    a_dst = np.asarray(a_dst, dtype=np.float32)
    bias = np.asarray(bias, dtype=np.float32)

    in_maps = []
    for c in range(NCORES):
        b = c // 2
        h0 = 2 * (c % 2)
        w2 = np.concatenate([w[h0], w[h0 + 1]], axis=1)          # [768, 128]
        asd = np.stack([a_src[h0, :, 0], a_dst[h0, :, 0],
                        a_src[h0 + 1, :, 0], a_dst[h0 + 1, :, 0]], axis=1)
        in_maps.append({
            "h": np.ascontiguousarray(h[b]),
            "w2": np.ascontiguousarray(w2),
            "asd": np.ascontiguousarray(asd),
            "biasc": np.ascontiguousarray(bias[:, None]),
        })

    res = run_bass_kernel_spmd(nc, in_maps, core_ids=list(range(NCORES)))

    out = np.empty((BS, N_HEAD, N, F_OUT), dtype=np.float32)
    for c in range(NCORES):
        b = c // 2
        h0 = 2 * (c % 2)
        oc = res.results[c]["o"]                                  # [2, 64, n]
        out[b, h0] = oc[0].T
        out[b, h0 + 1] = oc[1].T
    return out


# revision 5
# speedup vs baseline: 1.0694x; 1.0694x over previous
"""BatchMultiHeadGraphAttention Trainium2 kernel (8 NeuronCores, SPMD).

Reference computation (per batch b, head h):
    hp   = h[b] @ w[h]                      [n, 64]
    t    = tanh(hp)
    src  = t @ a_src[h];  dst = t @ a_dst[h]        [n]
    attn = softmax_j( leaky_relu(src_i + dst_j, 0.2) )
    out  = attn @ hp + bias

Key identity used here: with z = src_i + dst_j,
    exp(lrelu(z)) = max(exp(z), exp(0.2 z))            (monotonicity)
                  = e02s_i * e02d_j * max(q_i * r_j, 1)
with q = exp(0.8 src), r = exp(0.8 dst), e02d = exp(0.2 dst).
The e02s_i factor cancels in the softmax normalization, and e02d_j folds
into the matmul stationary operand, so the whole [n, n] attention matrix
costs ONE fused DVE pass (mult+max) per 128-row chunk:
    g[j, i]   = max(q_i * r_j, 1)                       (bf16)
    psum[o,i] = sum_j (e02d_j * (hp[j,o] + bias_o)) * g[j,i]   (PE, bf16)
    psum[64,i]= sum_j  e02d_j * g[j,i]                  (ones column)
    out_T[o,i]= psum[o,i] / psum[64,i]
bias is folded exactly: sum_j p*(hp+bias)/sum_j p = out + bias.

Sharding: 16 (b,h) pairs over 8 cores -> core c handles batch c//2,
heads {2*(c%2), 2*(c%2)+1}. Output returned transposed [64, n] per head;
the host does the final [o, n] -> [n, o] transpose during the gather.
"""

import numpy as np

import concourse.bass as bass
import concourse.mybir as mybir
from concourse.tile import TileContext
from concourse.bass_utils import run_bass_kernel_spmd
from concourse.masks import make_identity

F32 = mybir.dt.float32
F32R = mybir.dt.float32r
BF16 = mybir.dt.bfloat16
AF = mybir.ActivationFunctionType
ALU = mybir.AluOpType

N_HEAD, F_IN, F_OUT = 4, 768, 64
BS, N = 4, 2048
NCORES = 8
NCH = N // 128      # 16 n-chunks of 128
FCH = F_IN // 128   # 6 f-chunks of 128
NBL = N // 512      # 4 n-blocks of 512


def _split_multi_waits(nc):
    """walrus in this container accepts at most ONE sync wait per
    instruction; hoist extra waits onto same-engine NOPs."""
    k = 0
    for fn in nc.m.functions:
        for bb in fn.blocks:
            il = bb.instructions
            new_list = []
            for inst in il:
                si = inst.sync_info
                if si is not None and si.on_wait and len(si.on_wait) > 1:
                    waits = list(si.on_wait)
                    for w in waits[:-1]:
                        nop = mybir.InstNoOp(
                            name=f"waitsplit-{k}",
                            engine=inst.engine,
                            sync_info=mybir.SyncInfo(on_wait=[w], on_update=[]),
                            bass_nofuse=True,
                        )
                        k += 1
                        new_list.append(nop)
                    inst.sync_info = mybir.SyncInfo(
                        on_wait=[waits[-1]], on_update=list(si.on_update or [])
                    )
                new_list.append(inst)
            il[:] = new_list
    return k


def build_module():
    nc = bass.Bass("TRN2", target_bir_lowering=False, debug=False,
                   num_devices=NCORES)
    h_d = nc.dram_tensor("h", [N, F_IN], F32, kind="ExternalInput")
    w2_d = nc.dram_tensor("w2", [F_IN, 128], F32, kind="ExternalInput")
    asd_d = nc.dram_tensor("asd", [F_OUT, 4], F32, kind="ExternalInput")
    bias_d = nc.dram_tensor("biasc", [F_OUT, 1], F32, kind="ExternalInput")
    o_d = nc.dram_tensor("o", [2, F_OUT, N], F32, kind="ExternalOutput")

    with TileContext(nc, num_cores=NCORES) as tc:
        with (
            tc.tile_pool(name="const", bufs=1) as const,
            tc.tile_pool(name="big", bufs=1) as big,
            tc.tile_pool(name="hstage", bufs=2) as hstage,
            tc.tile_pool(name="gpool", bufs=3) as gpool,
            tc.tile_pool(name="dram", bufs=1, space="DRAM") as dram,
        ):
            ident = const.tile([128, 128], F32)
            make_identity(nc, ident)
            w2r = const.tile([128, FCH, 128], F32R)
            nc.sync.dma_start(
                w2r[:], w2_d[:].rearrange("(c p) m -> p c m", p=128).bitcast(F32R))
            asd_sb = const.tile([F_OUT, 4], F32)
            nc.sync.dma_start(asd_sb[:], asd_d[:])
            bias_sb = const.tile([F_OUT, 1], F32)
            nc.sync.dma_start(bias_sb[:], bias_d[:])

            # persistent SBUF
            hT = [big.tile([128, N], F32R, tag=f"hT{fc}", name=f"hT{fc}") for fc in range(FCH)]
            hpT = [big.tile([65, N], F32, tag=f"hpT{hh}", name=f"hpT{hh}") for hh in range(2)]
            tT = [big.tile([F_OUT, N], F32, tag=f"tT{hh}", name=f"tT{hh}") for hh in range(2)]

            # ---------- Phase A: load h and transpose to hT (f32r) ----------
            with tc.tile_pool(name="psA", bufs=4, space="PSUM") as psA, \
                 tc.tile_pool(name="psB", bufs=4, space="PSUM") as psB:
                for ncnk in range(NCH):
                    hs = hstage.tile([128, F_IN], F32)
                    nc.sync.dma_start(hs[:], h_d[ncnk * 128:(ncnk + 1) * 128, :])
                    for fc in range(FCH):
                        pt = psA.tile([128, 128], F32, tag="pt")
                        nc.tensor.transpose(
                            pt[:], hs[:, fc * 128:(fc + 1) * 128], ident[:])
                        dst = hT[fc][:, ncnk * 128:(ncnk + 1) * 128]
                        if (ncnk * FCH + fc) % 2 == 0:
                            nc.vector.tensor_copy(dst, pt[:])
                        else:
                            nc.scalar.activation(dst, pt[:], AF.Copy)

                # ---------- Phase B: hp_T for both heads, tanh, bias-fold ----
                for hh in range(2):
                    nc.gpsimd.memset(hpT[hh][64:65, :], 1.0)
                for nb in range(NBL):
                    pB = psB.tile([128, 512], F32, tag="pB")
                    for fc in range(FCH):
                        nc.tensor.matmul(
                            pB[:], w2r[:, fc, :],
                            hT[fc][:, nb * 512:(nb + 1) * 512],
                            start=(fc == 0), stop=(fc == FCH - 1))
                    nbs = slice(nb * 512, (nb + 1) * 512)
                    for hh in range(2):
                        rows = slice(hh * 64, hh * 64 + 64)
                        nc.scalar.activation(tT[hh][:, nbs], pB[rows, :], AF.Tanh)
                        nc.scalar.activation(hpT[hh][0:64, nbs], pB[rows, :],
                                             AF.Identity, bias=bias_sb[:])

            # ---------- per-head phases ----------
            # Order: C0 C1 D0 D1 | F0 | G0-copy F1 | G0-rest G1 — both heads'
            # PE matmul streams run back-to-back so the PE stays HAM-warm;
            # epilogues happen on ACT/DVE/DMA underneath F of the other head.
            with tc.tile_pool(name="psM", bufs=2, space="PSUM") as psM, \
                 tc.tile_pool(name="psOut", bufs=4, space="PSUM") as psOut:
                q_bcast, r_col, e02d, xs = [], [], [], []
                for hh in range(2):
                    # Phase C: src/dst rows via PE; q/r/e02d exps
                    sd_sb = big.tile([2, N], F32, tag="sd", name=f"sd{hh}")
                    for nb in range(NBL):
                        nbs = slice(nb * 512, (nb + 1) * 512)
                        pC = psM.tile([2, 512], F32, tag="psm", name=f"pC{hh}_{nb}")
                        nc.tensor.matmul(pC[:], asd_sb[:, 2 * hh:2 * hh + 2],
                                         tT[hh][:, nbs], start=True, stop=True)
                        nc.scalar.activation(sd_sb[:, nbs], pC[:], AF.Copy)
                    q_bf = big.tile([1, N], BF16, tag="qbf", name=f"q{hh}")
                    nc.scalar.activation(q_bf[:], sd_sb[0:1, :], AF.Exp, scale=0.8)
                    qd = dram.tile([1, N], BF16, tag=f"qd{hh}", name=f"qd{hh}")
                    nc.sync.dma_start(qd[:], q_bf[:])
                    qb = big.tile([128, N], BF16, tag=f"qb{hh}", name=f"qb{hh}")
                    nc.sync.dma_start(qb[:], qd[:].partition_broadcast(128))
                    q_bcast.append(qb)

                    pD = psM.tile([128, NCH, 2], F32, tag="psm", name=f"pD{hh}")
                    for k in range(NCH):
                        nc.tensor.transpose(pD[:, k, :],
                                            sd_sb[:, k * 128:(k + 1) * 128],
                                            ident[0:2, 0:2])
                    rc = big.tile([128, NCH], F32, tag=f"r{hh}", name=f"r{hh}")
                    nc.scalar.activation(rc[:], pD[:, :, 1], AF.Exp, scale=0.8)
                    r_col.append(rc)
                    ed = big.tile([128, NCH], F32, tag=f"e02d{hh}", name=f"e02d{hh}")
                    nc.scalar.activation(ed[:], pD[:, :, 1], AF.Exp, scale=0.2)
                    e02d.append(ed)

                    # Phase D: stationary tiles hpe02X = e02d * [hp + bias | 1]
                    xh = []
                    for jc in range(NCH):
                        pX = psM.tile([128, 65], F32, tag="psm", name=f"pX{hh}_{jc}")
                        nc.tensor.transpose(pX[:],
                                            hpT[hh][0:65, jc * 128:(jc + 1) * 128],
                                            ident[0:65, 0:65])
                        xt = big.tile([128, 65], BF16, tag=f"x{hh}_{jc}", name=f"x{hh}_{jc}")
                        nc.scalar.activation(xt[:], pX[:], AF.Copy,
                                             scale=ed[:, jc:jc + 1])
                        xh.append(xt)
                    xs.append(xh)

                # Phases F & G interleaved across heads
                outp = [big.tile([65, N], F32, tag=f"outp{hh}", name=f"outp{hh}")
                        for hh in range(2)]
                for hh in range(2):
                    pOut = []
                    for ib in range(NBL):
                        pOut.append(psOut.tile([65, 512], F32, tag="pout",
                                               name=f"pout{hh}_{ib}"))
                    for jc in range(NCH):
                        g = gpool.tile([128, N], BF16, tag="g", name=f"g{hh}_{jc}")
                        nc.vector.tensor_scalar(
                            out=g[:], in0=q_bcast[hh][:],
                            scalar1=r_col[hh][:, jc:jc + 1], scalar2=1.0,
                            op0=ALU.mult, op1=ALU.max)
                        for ib in range(NBL):
                            nc.tensor.matmul(
                                pOut[ib][:], xs[hh][jc][:],
                                g[:, ib * 512:(ib + 1) * 512],
                                start=(jc == 0), stop=(jc == NCH - 1))
                    # free PSUM banks ASAP: single copy to SBUF, rest of the
                    # epilogue runs from SBUF under the other head's F phase
                    for ib in range(NBL):
                        nbs = slice(ib * 512, (ib + 1) * 512)
                        nc.vector.tensor_copy(outp[hh][:, nbs], pOut[ib][:])

                for hh in range(2):
                    # Phase G: out = outp[0:64] / outp[64]  (recip via ACT ln/exp)
                    lnr = big.tile([1, N], F32, tag="lnr", name=f"ln{hh}")
                    nc.scalar.activation(lnr[:], outp[hh][64:65, :], AF.Ln)
                    rs = big.tile([1, N], F32, tag="rsr", name=f"rs{hh}")
                    nc.scalar.activation(rs[:], lnr[:], AF.Exp, scale=-1.0)
                    rsd = dram.tile([1, N], F32, tag=f"rsd{hh}", name=f"rsd{hh}")
                    nc.sync.dma_start(rsd[:], rs[:])
                    rs_bcast = big.tile([F_OUT, N], F32, tag=f"rsb{hh}", name=f"rsb{hh}")
                    nc.sync.dma_start(rs_bcast[:], rsd[:].partition_broadcast(F_OUT))
                    nc.vector.tensor_tensor(
                        out=outp[hh][0:64, :], in0=outp[hh][0:64, :],
                        in1=rs_bcast[:], op=ALU.mult)
                    nc.sync.dma_start(o_d[hh], outp[hh][0:64, :])

    _split_multi_waits(nc)
    return nc


_NC_CACHE = None


def kernel(h, w, a_src, a_dst, bias):
    global _NC_CACHE
    if _NC_CACHE is None:
        _NC_CACHE = build_module()
    nc = _NC_CACHE

    h = np.asarray(h, dtype=np.float32)
    w = np.asarray(w, dtype=np.float32)
    a_src = np.asarray(a_src, dtype=np.float32)
    a_dst = np.asarray(a_dst, dtype=np.float32)
    bias = np.asarray(bias, dtype=np.float32)

    in_maps = []
    for c in range(NCORES):
        b = c // 2
        h0 = 2 * (c % 2)
        w2 = np.concatenate([w[h0], w[h0 + 1]], axis=1)          # [768, 128]
        asd = np.stack([a_src[h0, :, 0], a_dst[h0, :, 0],
                        a_src[h0 + 1, :, 0], a_dst[h0 + 1, :, 0]], axis=1)
        in_maps.append({
            "h": np.ascontiguousarray(h[b]),
            "w2": np.ascontiguousarray(w2),
            "asd": np.ascontiguousarray(asd),
            "biasc": np.ascontiguousarray(bias[:, None]),
        })

    res = run_bass_kernel_spmd(nc, in_maps, core_ids=list(range(NCORES)))

    out = np.empty((BS, N_HEAD, N, F_OUT), dtype=np.float32)
    for c in range(NCORES):
        b = c // 2
        h0 = 2 * (c % 2)
        oc = res.results[c]["o"]                                  # [2, 64, n]
        out[b, h0] = oc[0].T
        out[b, h0 + 1] = oc[1].T
    return out


# revision 6
# speedup vs baseline: 1.2451x; 1.1642x over previous
"""BatchMultiHeadGraphAttention Trainium2 kernel (8 NeuronCores, SPMD).

Reference computation (per batch b, head h):
    hp   = h[b] @ w[h]                      [n, 64]
    t    = tanh(hp)
    src  = t @ a_src[h];  dst = t @ a_dst[h]        [n]
    attn = softmax_j( leaky_relu(src_i + dst_j, 0.2) )
    out  = attn @ hp + bias

Key identity used here: with z = src_i + dst_j,
    exp(lrelu(z)) = max(exp(z), exp(0.2 z))            (monotonicity)
                  = e02s_i * e02d_j * max(q_i * r_j, 1)
with q = exp(0.8 src), r = exp(0.8 dst), e02d = exp(0.2 dst).
The e02s_i factor cancels in the softmax normalization, and e02d_j folds
into the matmul stationary operand, so the whole [n, n] attention matrix
costs ONE fused DVE pass (mult+max) per 128-row chunk:
    g[j, i]   = max(q_i * r_j, 1)                       (bf16)
    psum[o,i] = sum_j (e02d_j * (hp[j,o] + bias_o)) * g[j,i]   (PE, bf16)
    psum[64,i]= sum_j  e02d_j * g[j,i]                  (ones column)
    out_T[o,i]= psum[o,i] / psum[64,i]
bias is folded exactly: sum_j p*(hp+bias)/sum_j p = out + bias.

Sharding: 16 (b,h) pairs over 8 cores -> core c handles batch c//2,
heads {2*(c%2), 2*(c%2)+1}. Output returned transposed [64, n] per head;
the host does the final [o, n] -> [n, o] transpose during the gather.
"""

import numpy as np

import concourse.bass as bass
import concourse.mybir as mybir
from concourse.tile import TileContext
from concourse.bass_utils import run_bass_kernel_spmd
from concourse.masks import make_identity

F32 = mybir.dt.float32
F32R = mybir.dt.float32r
BF16 = mybir.dt.bfloat16
AF = mybir.ActivationFunctionType
ALU = mybir.AluOpType

N_HEAD, F_IN, F_OUT = 4, 768, 64
BS, N = 4, 2048
NCORES = 8
NCH = N // 128      # 16 n-chunks of 128
FCH = F_IN // 128   # 6 f-chunks of 128
NBL = N // 512      # 4 n-blocks of 512


def _split_multi_waits(nc):
    """walrus in this container accepts at most ONE sync wait per
    instruction; hoist extra waits onto same-engine NOPs."""
    k = 0
    for fn in nc.m.functions:
        for bb in fn.blocks:
            il = bb.instructions
            new_list = []
            for inst in il:
                si = inst.sync_info
                if si is not None and si.on_wait and len(si.on_wait) > 1:
                    waits = list(si.on_wait)
                    for w in waits[:-1]:
                        nop = mybir.InstNoOp(
                            name=f"waitsplit-{k}",
                            engine=inst.engine,
                            sync_info=mybir.SyncInfo(on_wait=[w], on_update=[]),
                            bass_nofuse=True,
                        )
                        k += 1
                        new_list.append(nop)
                    inst.sync_info = mybir.SyncInfo(
                        on_wait=[waits[-1]], on_update=list(si.on_update or [])
                    )
                new_list.append(inst)
            il[:] = new_list
    return k


def build_module():
    nc = bass.Bass("TRN2", target_bir_lowering=False, debug=False,
                   num_devices=NCORES)
    h_d = nc.dram_tensor("h", [N, F_IN], F32, kind="ExternalInput")
    w2_d = nc.dram_tensor("w2", [F_IN, 128], F32, kind="ExternalInput")
    asd_d = nc.dram_tensor("asd", [F_OUT, 4], F32, kind="ExternalInput")
    bias_d = nc.dram_tensor("biasc", [F_OUT, 1], F32, kind="ExternalInput")
    o_d = nc.dram_tensor("o", [2, F_OUT, N], F32, kind="ExternalOutput")

    with TileContext(nc, num_cores=NCORES) as tc:
        with (
            tc.tile_pool(name="const", bufs=1) as const,
            tc.tile_pool(name="big", bufs=1) as big,
            tc.tile_pool(name="hstage", bufs=3) as hstage,
            tc.tile_pool(name="gpool", bufs=3) as gpool,
            tc.tile_pool(name="dram", bufs=1, space="DRAM") as dram,
        ):
            ident = const.tile([128, 128], F32)
            make_identity(nc, ident)
            w2r = const.tile([128, FCH, 128], F32R)
            nc.scalar.dma_start(
                w2r[:], w2_d[:].rearrange("(c p) m -> p c m", p=128).bitcast(F32R))
            asd_sb = const.tile([F_OUT, 4], F32)
            nc.scalar.dma_start(asd_sb[:], asd_d[:])
            bias_sb = const.tile([F_OUT, 1], F32)
            nc.scalar.dma_start(bias_sb[:], bias_d[:])

            # persistent SBUF; hT is tiled [fc][nb] -> [128, 512] so phase B
            # can start on an n-block as soon as its 4 n-chunks are transposed
            hT = [[big.tile([128, 512], F32R, tag=f"hT{fc}_{nb}",
                            name=f"hT{fc}_{nb}") for nb in range(NBL)]
                  for fc in range(FCH)]
            hpT = [big.tile([65, N], F32, tag=f"hpT{hh}", name=f"hpT{hh}")
                   for hh in range(2)]
            tT = [big.tile([F_OUT, N], F32, tag=f"tT{hh}", name=f"tT{hh}")
                  for hh in range(2)]

            # ---------- Phase A: load h, transpose to hT (f32r) ----------
            # ---------- Phase B: hp_T per n-block as soon as ready --------
            with tc.tile_pool(name="psA", bufs=4, space="PSUM") as psA, \
                 tc.tile_pool(name="psB", bufs=4, space="PSUM") as psB:
                for hh in range(2):
                    nc.gpsimd.memset(hpT[hh][64:65, :], 1.0)
                cb = 0  # copyback round-robin
                for nb in range(NBL):
                    for sub in range(4):          # 4 n-chunks per block
                        ncnk = nb * 4 + sub
                        hs = hstage.tile([128, F_IN], F32, tag="hs",
                                         name=f"hs{ncnk}")
                        nc.sync.dma_start(hs[:],
                                          h_d[ncnk * 128:(ncnk + 1) * 128, :])
                        for fc in range(FCH):
                            pt = psA.tile([128, 128], F32, tag="pt",
                                          name=f"pt{ncnk}_{fc}")
                            nc.tensor.transpose(
                                pt[:], hs[:, fc * 128:(fc + 1) * 128], ident[:])
                            dst = hT[fc][nb][:, sub * 128:(sub + 1) * 128]
                            if cb % 5 < 3:
                                nc.scalar.activation(dst, pt[:], AF.Copy)
                            else:
                                nc.vector.tensor_copy(dst, pt[:])
                            cb += 1
                    # hp_T for this n-block (both heads at once, M=128)
                    pB = psB.tile([128, 512], F32, tag="pB", name=f"pB{nb}")
                    for fc in range(FCH):
                        nc.tensor.matmul(
                            pB[:], w2r[:, fc, :], hT[fc][nb][:],
                            start=(fc == 0), stop=(fc == FCH - 1))
                    nbs = slice(nb * 512, (nb + 1) * 512)
                    for hh in range(2):
                        rows = slice(hh * 64, hh * 64 + 64)
                        nc.scalar.activation(tT[hh][:, nbs], pB[rows, :],
                                             AF.Tanh)
                        nc.vector.tensor_scalar_add(hpT[hh][0:64, nbs],
                                                    pB[rows, :], bias_sb[:])

            # ---------- per-head phases ----------
            # Order: C0 C1 D0 D1 | F0 | G0 under F1 | G1.
            with tc.tile_pool(name="psM", bufs=4, space="PSUM") as psM, \
                 tc.tile_pool(name="psOut", bufs=4, space="PSUM") as psOut:
                q_bcast, r_col, e02d, xs = [], [], [], []
                for hh in range(2):
                    # Phase C: src/dst rows via PE; q/r/e02d exps
                    sd_sb = big.tile([2, N], F32, tag="sd", name=f"sd{hh}")
                    for nb in range(NBL):
                        nbs = slice(nb * 512, (nb + 1) * 512)
                        pC = psM.tile([2, 512], F32, tag="psm",
                                      name=f"pC{hh}_{nb}")
                        nc.tensor.matmul(pC[:], asd_sb[:, 2 * hh:2 * hh + 2],
                                         tT[hh][:, nbs], start=True, stop=True)
                        nc.vector.tensor_copy(sd_sb[:, nbs], pC[:])
                    q_bf = big.tile([1, N], BF16, tag="qbf", name=f"q{hh}")
                    nc.scalar.activation(q_bf[:], sd_sb[0:1, :], AF.Exp,
                                         scale=0.8)
                    qd = dram.tile([1, N], BF16, tag=f"qd{hh}", name=f"qd{hh}")
                    nc.scalar.dma_start(qd[:], q_bf[:])
                    qb = big.tile([128, N], BF16, tag=f"qb{hh}", name=f"qb{hh}")
                    nc.scalar.dma_start(qb[:], qd[:].partition_broadcast(128))
                    q_bcast.append(qb)

                    pD = psM.tile([128, NCH, 2], F32, tag="psm", name=f"pD{hh}")
                    for k in range(NCH):
                        nc.tensor.transpose(pD[:, k, :],
                                            sd_sb[:, k * 128:(k + 1) * 128],
                                            ident[0:2, 0:2])
                    rc = big.tile([128, NCH], F32, tag=f"r{hh}", name=f"r{hh}")
                    nc.scalar.activation(rc[:], pD[:, :, 1], AF.Exp, scale=0.8)
                    r_col.append(rc)
                    ed = big.tile([128, NCH], F32, tag=f"e02d{hh}",
                                  name=f"e02d{hh}")
                    nc.scalar.activation(ed[:], pD[:, :, 1], AF.Exp, scale=0.2)
                    e02d.append(ed)

                    # Phase D: stationary tiles hpe02X = e02d * [hp + bias | 1]
                    xh = []
                    for jc in range(NCH):
                        pX = psM.tile([128, 65], F32, tag="psm",
                                      name=f"pX{hh}_{jc}")
                        nc.tensor.transpose(pX[:],
                                            hpT[hh][0:65,
                                                    jc * 128:(jc + 1) * 128],
                                            ident[0:65, 0:65])
                        xt = big.tile([128, 65], BF16, tag=f"x{hh}_{jc}",
                                      name=f"x{hh}_{jc}")
                        nc.vector.tensor_scalar_mul(xt[:], pX[:],
                                                    ed[:, jc:jc + 1])
                        xh.append(xt)
                    xs.append(xh)

                # Phases F & G, G(hh) running under F(hh+1)
                outp = [big.tile([65, N], F32, tag=f"outp{hh}",
                                 name=f"outp{hh}") for hh in range(2)]
                for hh in range(2):
                    pOut = []
                    for ib in range(NBL):
                        pOut.append(psOut.tile([65, 512], F32, tag="pout",
                                               name=f"pout{hh}_{ib}"))
                    for jc in range(NCH):
                        g = gpool.tile([128, N], BF16, tag="g",
                                       name=f"g{hh}_{jc}")
                        nc.vector.tensor_scalar(
                            out=g[:], in0=q_bcast[hh][:],
                            scalar1=r_col[hh][:, jc:jc + 1], scalar2=1.0,
                            op0=ALU.mult, op1=ALU.max)
                        for ib in range(NBL):
                            nc.tensor.matmul(
                                pOut[ib][:], xs[hh][jc][:],
                                g[:, ib * 512:(ib + 1) * 512],
                                start=(jc == 0), stop=(jc == NCH - 1))
                    # free PSUM banks ASAP (ACT is idle during F; DVE makes g)
                    for ib in range(NBL):
                        nbs = slice(ib * 512, (ib + 1) * 512)
                        nc.scalar.activation(outp[hh][:, nbs], pOut[ib][:],
                                             AF.Copy)

                    # Phase G, pipelined per 512-block:
                    # ln -> exp(-1) -> DRAM roundtrip bcast -> mult -> store
                    lnr = big.tile([1, N], F32, tag="lnr", name=f"ln{hh}")
                    rs = big.tile([1, N], F32, tag="rsr", name=f"rs{hh}")
                    rsd = dram.tile([1, N], F32, tag=f"rsd{hh}",
                                    name=f"rsd{hh}")
                    rs_bcast = big.tile([F_OUT, N], F32, tag=f"rsb{hh}",
                                        name=f"rsb{hh}")
                    for ib in range(NBL):
                        nbs = slice(ib * 512, (ib + 1) * 512)
                        nc.scalar.activation(lnr[0:1, nbs],
                                             outp[hh][64:65, nbs], AF.Ln)
                        nc.scalar.activation(rs[0:1, nbs], lnr[0:1, nbs],
                                             AF.Exp, scale=-1.0)
                        nc.scalar.dma_start(rsd[0:1, nbs], rs[0:1, nbs])
                        nc.scalar.dma_start(
                            rs_bcast[:, nbs],
                            rsd[0:1, nbs].partition_broadcast(F_OUT))
                        nc.vector.tensor_tensor(
                            out=outp[hh][0:64, nbs], in0=outp[hh][0:64, nbs],
                            in1=rs_bcast[:, nbs], op=ALU.mult)
                        nc.sync.dma_start(o_d[hh][:, nbs],
                                          outp[hh][0:64, nbs])

    _split_multi_waits(nc)
    return nc


_NC_CACHE = None


def kernel(h, w, a_src, a_dst, bias):
    global _NC_CACHE
    if _NC_CACHE is None:
        _NC_CACHE = build_module()
    nc = _NC_CACHE

    h = np.asarray(h, dtype=np.float32)
    w = np.asarray(w, dtype=np.float32)
    a_src = np.asarray(a_src, dtype=np.float32)
    a_dst = np.asarray(a_dst, dtype=np.float32)
    bias = np.asarray(bias, dtype=np.float32)

    in_maps = []
    for c in range(NCORES):
        b = c // 2
        h0 = 2 * (c % 2)
        w2 = np.concatenate([w[h0], w[h0 + 1]], axis=1)          # [768, 128]
        asd = np.stack([a_src[h0, :, 0], a_dst[h0, :, 0],
                        a_src[h0 + 1, :, 0], a_dst[h0 + 1, :, 0]], axis=1)
        in_maps.append({
            "h": np.ascontiguousarray(h[b]),
            "w2": np.ascontiguousarray(w2),
            "asd": np.ascontiguousarray(asd),
            "biasc": np.ascontiguousarray(bias[:, None]),
        })

    res = run_bass_kernel_spmd(nc, in_maps, core_ids=list(range(NCORES)))

    out = np.empty((BS, N_HEAD, N, F_OUT), dtype=np.float32)
    for c in range(NCORES):
        b = c // 2
        h0 = 2 * (c % 2)
        oc = res.results[c]["o"]                                  # [2, 64, n]
        out[b, h0] = oc[0].T
        out[b, h0 + 1] = oc[1].T
    return out
